# revision 10
# baseline (speedup 1.0000x reference)
"""Trainium2 Bass kernel for a 2-layer edge-weighted GraphSAGE network.

Strategy (8 NeuronCores, dst-sharded, SPMD-uniform program):
  * Host converts the edge list (src, dst, w) into the dense row-normalized
    adjacency operator A[d, s] = sum_e w_e / max(deg_d, 1); each layer's
    weighted segment-mean becomes a dense matmul hN = A @ h.  Src rows are
    padded per-core to 1280 (10 blocks of 128) so the 8 dst-shards all see
    the same 80-block layout; A^T is stored fp8e4m3 scaled by 64, striped
    over the three DMA-capable queues, and stays resident in SBUF for
    reuse by layer 2.
  * Layer 2's aggregated half is folded through W2's bottom half first --
    xN @ W2b == A @ (x @ W2b) -- so each core all-gathers y = x @ W2b
    ([1250, 64] fp8) instead of the full x: 4x less collective traffic,
    half the layer-2 PE time, and no transposes anywhere (y is computed
    directly in row-major via x^T-stationary matmuls).
  * Both A-sweeps run fp8 DoubleRow (2 MACs/cycle): h is quantized to
    fp8e4m3 for the layer-1 sweep (the f16 local copy h^T still feeds the
    concat-linear exactly).
  * The post-L1 chain is pipelined per 512-col dst chunk, and y is
    all-gathered in two column chunks (blocks 0-3, then 4-9) so the
    layer-2 sweep starts as soon as the first chunk lands while the
    second is still in flight.
  * Two back-to-back warm-up AllGathers fire at t~0 on gpsimd: the first
    absorbs the one-time collective-init cost (~50us!), the second the
    per-collective rendezvous, so the real all-gathers start immediately.
  * Scale bookkeeping: A carries x64 (fp8 range), W1's hN-half carries
    /64, W2a carries x64, undone in the final PSUM->SBUF copy.
"""

import os
import sys
import types

sys.path.insert(0, "/opt/trn_rl_repo")

import numpy as np

import concourse.bacc as bacc
import concourse.tile as tile
from concourse import mybir
from concourse import bass_utils

N_NODES = 10000
N_EDGES = 640000
D_IN, D_HID, D_OUT = 128, 256, 64
N_CORES = 8
P = 128
NB = N_NODES // N_CORES          # 1250 local dst nodes per core
BPC = 10                         # padded src blocks per core region
KB = N_CORES * BPC               # 80 global src blocks (10240 padded rows)
NPAD = KB * P
GPAIRS = KB // 2                 # 40 DoubleRow pairs
ASCALE = 64.0                    # fp8 pre-scale on A (undone downstream)
F8 = mybir.dt.float8e4
F16 = mybir.dt.float16
F32 = mybir.dt.float32

# free-axis chunks of the local dst range (PSUM bank = 512 f32);
# chunk i covers dst blocks CH_BLK[i]
N_CHUNKS = [(0, 512), (512, 1024), (1024, NB)]
CH_BLK = [(0, 4), (4, 8), (8, 10)]
# y all-gather split: chunk a = blocks 0-3, chunk b = blocks 4-9
AG_SPLIT = 4 * D_OUT             # 256 cols of y8l
A_PAIRS = [g for g in range(GPAIRS) if g % 5 < 2]   # pairs served by AG a
B_PAIRS = [g for g in range(GPAIRS) if g % 5 >= 2]  # pairs served by AG b

_compiled_nc = None
LAST_EXEC_NS = None


def _build_nc():
    nc = bacc.Bacc("TRN2", target_bir_lowering=False, debug=False,
                   num_devices=N_CORES)

    as_d = nc.dram_tensor("as8", [GPAIRS, P, 2 * NB], F8, kind="ExternalInput")
    hs_d = nc.dram_tensor("hsb8", [P, KB * D_IN], F8, kind="ExternalInput")
    ht_d = nc.dram_tensor("ht", [D_IN, NB], F16, kind="ExternalInput")
    w1_d = nc.dram_tensor("w1", [2 * D_IN, D_HID], F16, kind="ExternalInput")
    w2a_d = nc.dram_tensor("w2a", [D_HID, D_OUT], F16, kind="ExternalInput")
    w2b_d = nc.dram_tensor("w2b", [D_HID, D_OUT], F16, kind="ExternalInput")
    b1_d = nc.dram_tensor("b1c", [P, 2], F32, kind="ExternalInput")
    b2_d = nc.dram_tensor("b2c", [D_OUT, 1], F32, kind="ExternalInput")
    out_d = nc.dram_tensor("outT", [D_OUT, NB], F32, kind="ExternalOutput")

    with tile.TileContext(nc) as tc:
        with (
            tc.tile_pool(name="const", bufs=1) as cpool,
            tc.tile_pool(name="acache", bufs=1) as acpool,
            tc.tile_pool(name="work", bufs=1) as wpool,
            tc.tile_pool(name="dram", bufs=1, space="DRAM") as dpool,
        ):
            # ---- warm-up collectives at t~0 (gpsimd-only chain): the first
            # absorbs the one-time collective-engine init, the second the
            # per-collective rendezvous, so the real all-gathers below start
            # within ~1us of their trigger.
            warm_sb = cpool.tile([1, 32], F8)
            nc.gpsimd.memset(warm_sb[:], 0.0)
            warm_in = dpool.tile([1, 32], F8)
            warm_out = dpool.tile([N_CORES, 32], F8, addr_space="Shared")
            warm_out2 = dpool.tile([N_CORES, 32], F8, addr_space="Shared")
            nc.gpsimd.dma_start(out=warm_in[:], in_=warm_sb[:])
            for wo in (warm_out, warm_out2):
                nc.gpsimd.collective_compute(
                    "AllGather", mybir.AluOpType.bypass,
                    replica_groups=[list(range(N_CORES))],
                    ins=[warm_in[:]], outs=[wo[:]])

            # ---- resident loads ------------------------------------------
            hsb = cpool.tile([P, KB * D_IN], F8)
            HC = KB * D_IN // 4
            for j in range(4):
                nc.gpsimd.dma_start(out=hsb[:, j * HC:(j + 1) * HC],
                                    in_=hs_d[:, j * HC:(j + 1) * HC])
            hts = cpool.tile([P, NB], F16)
            nc.gpsimd.dma_start(out=hts[:], in_=ht_d[:])
            w1s = cpool.tile([P, 2 * D_HID], F16)
            for k in range(2):
                nc.gpsimd.dma_start(out=w1s[:, k * D_HID:(k + 1) * D_HID],
                                    in_=w1_d[k * P:(k + 1) * P, :])
            w2as = cpool.tile([P, 2 * D_OUT], F16)
            w2bs = cpool.tile([P, 2 * D_OUT], F16)
            for k in range(2):
                nc.gpsimd.dma_start(out=w2as[:, k * D_OUT:(k + 1) * D_OUT],
                                    in_=w2a_d[k * P:(k + 1) * P, :])
                nc.gpsimd.dma_start(out=w2bs[:, k * D_OUT:(k + 1) * D_OUT],
                                    in_=w2b_d[k * P:(k + 1) * P, :])
            b1s = cpool.tile([P, 2], F32)
            nc.gpsimd.dma_start(out=b1s[:], in_=b1_d[:])
            b2s = cpool.tile([D_OUT, 1], F32)
            nc.gpsimd.dma_start(out=b2s[:], in_=b2_d[:])

            # A^T stream: 40 pair tiles of [128, 2*1250] fp8, striped over
            # all three DMA-capable queues so delivery tracks the k-sweep.
            acq = [acpool.tile([P, 2 * NB], F8, name=f"acq{q}")
                   for q in range(GPAIRS)]
            for q in range(GPAIRS):
                eng = (nc.sync, nc.scalar, nc.gpsimd)[q % 3]
                eng.dma_start(out=acq[q][:], in_=as_d[q])

            def a_pair(q):
                return acq[q][:].rearrange("p (two d) -> p two d", two=2)

            hNT = wpool.tile([P, NB], F16)
            # xT padded to 1280 cols; the 30-col tail is zeroed once so the
            # ragged last y-block can matmul a full 128-wide stationary.
            xT = [wpool.tile([P, BPC * P], F16, name=f"xT{m}") for m in range(2)]
            for m in range(2):
                nc.vector.memset(xT[m][:, NB:], 0.0)
            y8l = wpool.tile([P, BPC * D_OUT], F8)
            y8g = wpool.tile([P, KB * D_OUT], F8)
            outsb = wpool.tile([D_OUT, NB], F32)

            # ---- layer 1 aggregation: hN^T = sum_k hk^T . As_k (x64) -----
            # fp8 DoubleRow over all 40 src pairs
            with tc.tile_pool(name="ps1", bufs=1, space="PSUM") as ps1:
                hN_ps = ps1.tile([P, NB], F32, space="PSUM")
                for q in range(GPAIRS):
                    lhs_pair = hsb[:, q * 2 * D_IN:(q + 1) * 2 * D_IN] \
                        .rearrange("p (two f) -> p two f", two=2)
                    rhs_pair = a_pair(q)
                    for (n0, n1) in N_CHUNKS:
                        nc.tensor.matmul(
                            out=hN_ps[:, n0:n1],
                            lhsT=lhs_pair,
                            rhs=rhs_pair[:, :, n0:n1],
                            perf_mode=mybir.MatmulPerfMode.DoubleRow,
                            start=(q == 0), stop=(q == GPAIRS - 1))
                # pure copy: the 1/64 is folded into W1's hN-half on host
                for i, (n0, n1) in enumerate(N_CHUNKS):
                    if i == 1:
                        nc.vector.tensor_copy(out=hNT[:, n0:n1],
                                              in_=hN_ps[:, n0:n1])
                    else:
                        nc.scalar.activation(
                            out=hNT[:, n0:n1], in_=hN_ps[:, n0:n1],
                            func=mybir.ActivationFunctionType.Copy)

            # ---- per-chunk pipelined chain: linear -> relu -> y -> AG ----
            # x^T = relu(W1^T . [h; hN]^T + b1); y = x @ W2b row-major;
            # o_ps also picks up its pre-AG W2a part per chunk.
            cat1 = [hts, hNT]
            ag_in_a = dpool.tile([P, AG_SPLIT], F8)
            ag_in_b = dpool.tile([P, BPC * D_OUT - AG_SPLIT], F8)
            ag_out_a = dpool.tile([N_CORES * P, AG_SPLIT], F8,
                                  addr_space="Shared")
            ag_out_b = dpool.tile([N_CORES * P, BPC * D_OUT - AG_SPLIT], F8,
                                  addr_space="Shared")

            ps2_ctx = tc.tile_pool(name="ps2", bufs=2, space="PSUM")
            ps2 = ps2_ctx.__enter__()
            ps3_ctx = tc.tile_pool(name="ps3", bufs=1, space="PSUM")
            ps3 = ps3_ctx.__enter__()
            ps4_ctx = tc.tile_pool(name="ps4", bufs=1, space="PSUM")
            ps4 = ps4_ctx.__enter__()
            o_ps = ps4.tile([D_OUT, NB], F32, space="PSUM")

            for ci, (n0, n1) in enumerate(N_CHUNKS):
                cw = n1 - n0
                # layer-1 linear for this chunk (m halves side by side in
                # one 2-bank ring tile)
                y_ps = ps2.tile([P, 1024], F32, space="PSUM", name="y_ps")
                for m in range(2):
                    for k in range(2):
                        nc.tensor.matmul(
                            out=y_ps[:, m * 512:m * 512 + cw],
                            lhsT=w1s[:, k * D_HID + m * P:
                                     k * D_HID + (m + 1) * P],
                            rhs=cat1[k][:, n0:n1],
                            start=(k == 0), stop=(k == 1))
                for m in range(2):
                    nc.scalar.activation(
                        out=xT[m][:, n0:n1],
                        in_=y_ps[:, m * 512:m * 512 + cw],
                        func=mybir.ActivationFunctionType.Relu,
                        bias=b1s[:, m:m + 1])
                # y blocks of this chunk (row-major, x^T-stationary)
                b0, b1_ = CH_BLK[ci]
                y_ps2 = ps3.tile([P, 4 * D_OUT], F32, space="PSUM",
                                 name="y_ps2")
                for bi, b in enumerate(range(b0, b1_)):
                    for m in range(2):
                        nc.tensor.matmul(
                            out=y_ps2[:, bi * D_OUT:(bi + 1) * D_OUT],
                            lhsT=xT[m][:, b * P:(b + 1) * P],
                            rhs=w2bs[:, m * D_OUT:(m + 1) * D_OUT],
                            start=(m == 0), stop=(m == 1))
                nc.vector.tensor_copy(
                    out=y8l[:, b0 * D_OUT:b1_ * D_OUT],
                    in_=y_ps2[:, :(b1_ - b0) * D_OUT])
                # pre-AG W2a part of the output (W2a pre-scaled x64)
                for m in range(2):
                    nc.tensor.matmul(
                        out=o_ps[:, n0:n1],
                        lhsT=w2as[:, m * D_OUT:(m + 1) * D_OUT],
                        rhs=xT[m][:, n0:n1],
                        start=(m == 0), stop=False)
                # kick off each all-gather as soon as its y columns exist
                if ci == 0:
                    nc.gpsimd.dma_start(out=ag_in_a[:],
                                        in_=y8l[:, :AG_SPLIT])
                    nc.gpsimd.collective_compute(
                        "AllGather", mybir.AluOpType.bypass,
                        replica_groups=[list(range(N_CORES))],
                        ins=[ag_in_a[:]], outs=[ag_out_a[:]])
                elif ci == 2:
                    nc.gpsimd.dma_start(out=ag_in_b[:],
                                        in_=y8l[:, AG_SPLIT:])
                    nc.gpsimd.collective_compute(
                        "AllGather", mybir.AluOpType.bypass,
                        replica_groups=[list(range(N_CORES))],
                        ins=[ag_in_b[:]], outs=[ag_out_b[:]])

            # ---- layer 2 sweep: out^T += sum_k yk^T . As_k (x64) ---------
            # a-pairs (y blocks 0-3 of every region) run while AG b is
            # still in flight.
            for c in range(N_CORES):
                nc.sync.dma_start(
                    out=y8g[:, c * BPC * D_OUT:c * BPC * D_OUT + AG_SPLIT],
                    in_=ag_out_a[c * P:(c + 1) * P, :])
            for c in range(N_CORES):
                nc.scalar.dma_start(
                    out=y8g[:, c * BPC * D_OUT + AG_SPLIT:
                            (c + 1) * BPC * D_OUT],
                    in_=ag_out_b[c * P:(c + 1) * P, :])
            for g in A_PAIRS + B_PAIRS:
                lhs_pair = y8g[:, g * 2 * D_OUT:(g + 1) * 2 * D_OUT] \
                    .rearrange("p (two f) -> p two f", two=2)
                rhs_pair = a_pair(g)
                for (n0, n1) in N_CHUNKS:
                    nc.tensor.matmul(
                        out=o_ps[:, n0:n1],
                        lhsT=lhs_pair,
                        rhs=rhs_pair[:, :, n0:n1],
                        perf_mode=mybir.MatmulPerfMode.DoubleRow,
                        start=False, stop=(g == B_PAIRS[-1]))
            for (n0, n1) in N_CHUNKS:
                nc.scalar.activation(out=outsb[:, n0:n1],
                                     in_=o_ps[:, n0:n1],
                                     func=mybir.ActivationFunctionType.Identity,
                                     bias=b2s[:, 0:1],
                                     scale=1.0 / ASCALE)
                nc.sync.dma_start(out=out_d[:, n0:n1],
                                  in_=outsb[:, n0:n1])

            ps4_ctx.__exit__(None, None, None)
            ps3_ctx.__exit__(None, None, None)
            ps2_ctx.__exit__(None, None, None)

    nc.compile()
    return nc


def _get_nc():
    global _compiled_nc
    if _compiled_nc is None:
        _compiled_nc = _build_nc()
    return _compiled_nc


def _enable_profile_hook():
    """Register the NTFF profiling hook that trn_boot skips when the image's
    antenv lacks axon_hooks (profiling only; used when GNN_PROFILE=1)."""
    try:
        import antenv
        if "antenv.axon_hooks" not in sys.modules:
            mod = types.ModuleType("antenv.axon_hooks")
            _h = [None]
            mod.set_axon_ntff_profile_hook = lambda hook: _h.__setitem__(0, hook)
            mod.get_axon_ntff_profile_hook = lambda: _h[0]
            sys.modules["antenv.axon_hooks"] = mod
            antenv.axon_hooks = mod
        from trn_agent_boot.trn_boot import _ntff_profile_via_ctypes
        hook = _ntff_profile_via_ctypes("/opt/axon/libaxon_pjrt.so")
        if hook is not None:
            sys.modules["antenv.axon_hooks"].set_axon_ntff_profile_hook(hook)
            return True
    except Exception:
        pass
    return False


def _host_prep(h, w, src, dst, W1, b1, W2, b2):
    import ml_dtypes
    import scipy.sparse as sp
    deg = np.bincount(dst, minlength=N_NODES).astype(np.float32)
    w_norm = (w[:, 0] * (ASCALE / np.maximum(deg, 1.0)[dst])).astype(np.float32)
    # AT[s, d] = sum of scaled w_norm over edges (s -> d): 64*A^T
    AT = sp.coo_matrix((w_norm, (src, dst)), shape=(N_NODES, N_NODES)).toarray()

    # per-core-padded src layout: row c*1280 + j <-> node c*1250 + j (j<1250)
    ATp = np.zeros((NPAD, N_NODES), dtype=np.float32)
    hp = np.zeros((NPAD, D_IN), dtype=np.float32)
    for c in range(N_CORES):
        ATp[c * BPC * P: c * BPC * P + NB] = AT[c * NB:(c + 1) * NB]
        hp[c * BPC * P: c * BPC * P + NB] = h[c * NB:(c + 1) * NB]
    AT8 = ATp.astype(ml_dtypes.float8_e4m3)
    hp8 = hp.astype(ml_dtypes.float8_e4m3)

    # hsb8[p, k*128+f] = hp8[k*128+p, f] (global padded block order)
    hsb8 = np.ascontiguousarray(
        hp8.reshape(KB, P, D_IN).transpose(1, 0, 2).reshape(P, KB * D_IN))

    w1c = W1.astype(np.float32).copy()
    w1c[D_IN:] *= 1.0 / ASCALE        # hN carries x64 out of its PSUM copy
    w1c = w1c.astype(np.float16)
    w2a = (W2[:D_HID] * ASCALE).astype(np.float16)
    w2b = W2[D_HID:].astype(np.float16)
    b1c = np.ascontiguousarray(b1.reshape(2, P).T.astype(np.float32))
    b2c = b2.reshape(D_OUT, 1).astype(np.float32)

    in_maps = []
    for c in range(N_CORES):
        ATc = AT8[:, c * NB:(c + 1) * NB]
        # as8[q, p, j*NB+d] = ATc[(2q+j)*128+p, d] (pair-interleaved)
        as8 = np.ascontiguousarray(
            ATc.reshape(GPAIRS, 2, P, NB).transpose(0, 2, 1, 3)
            .reshape(GPAIRS, P, 2 * NB))
        in_maps.append({
            "as8": as8,
            "hsb8": hsb8,
            "ht": np.ascontiguousarray(
                h[c * NB:(c + 1) * NB].T.astype(np.float16)),
            "w1": w1c,
            "w2a": w2a,
            "w2b": w2b,
            "b1c": b1c,
            "b2c": b2c,
        })
    return in_maps


def kernel(h, w, src, dst, W1, b1, W2, b2):
    global LAST_EXEC_NS
    h = np.asarray(h, dtype=np.float32)
    w = np.asarray(w, dtype=np.float32)
    src = np.asarray(src)
    dst = np.asarray(dst)
    W1 = np.asarray(W1, dtype=np.float32)
    b1 = np.asarray(b1, dtype=np.float32)
    W2 = np.asarray(W2, dtype=np.float32)
    b2 = np.asarray(b2, dtype=np.float32)

    in_maps = _host_prep(h, w, src, dst, W1, b1, W2, b2)
    nc = _get_nc()
    trace = os.environ.get("GNN_PROFILE") == "1" and _enable_profile_hook()
    res = bass_utils.run_bass_kernel_spmd(
        nc, in_maps, core_ids=list(range(N_CORES)), trace=trace)
    LAST_EXEC_NS = res.exec_time_ns

    out = np.concatenate(
        [res.results[c]["outT"].T for c in range(N_CORES)], axis=0)
    return out.astype(np.float32)


# revision 11
# speedup vs baseline: 1.0857x; 1.0857x over previous
"""Trainium2 Bass kernel for a 2-layer edge-weighted GraphSAGE network.

Strategy (8 NeuronCores, dst-sharded, SPMD-uniform program):
  * Host converts the edge list (src, dst, w) into the dense row-normalized
    adjacency operator A[d, s] = sum_e w_e / max(deg_d, 1); each layer's
    weighted segment-mean becomes a dense matmul hN = A @ h.  Src rows are
    padded per-core to 1280 (10 blocks of 128) so the 8 dst-shards all see
    the same 80-block layout; A^T is stored fp8e4m3 scaled by 64, striped
    over the three DMA-capable queues, and stays resident in SBUF for
    reuse by layer 2.
  * Layer 2's aggregated half is folded through W2's bottom half first --
    xN @ W2b == A @ (x @ W2b) -- so each core all-gathers y = x @ W2b
    ([1250, 64] fp8) instead of the full x: 4x less collective traffic,
    half the layer-2 PE time, and no transposes anywhere (y is computed
    directly in row-major via x^T-stationary matmuls).
  * Both A-sweeps run fp8 DoubleRow (2 MACs/cycle): h is quantized to
    fp8e4m3 for the layer-1 sweep (the f16 local copy h^T still feeds the
    concat-linear exactly).
  * The post-L1 chain is pipelined per 512-col dst chunk, and y is
    all-gathered in two column chunks (blocks 0-3, then 4-9) so the
    layer-2 sweep starts as soon as the first chunk lands while the
    second is still in flight.
  * Two back-to-back warm-up AllGathers fire at t~0 on gpsimd: the first
    absorbs the one-time collective-init cost (~50us!), the second the
    per-collective rendezvous, so the real all-gathers start immediately.
  * Scale bookkeeping: A carries x64 (fp8 range), W1's hN-half carries
    /64, W2a carries x64, undone in the final PSUM->SBUF copy.
"""

import os
import sys
import types

sys.path.insert(0, "/opt/trn_rl_repo")

import numpy as np

import concourse.bacc as bacc
import concourse.tile as tile
from concourse import mybir
from concourse import bass_utils

N_NODES = 10000
N_EDGES = 640000
D_IN, D_HID, D_OUT = 128, 256, 64
N_CORES = 8
P = 128
NB = N_NODES // N_CORES          # 1250 local dst nodes per core
BPC = 10                         # padded src blocks per core region
KB = N_CORES * BPC               # 80 global src blocks (10240 padded rows)
NPAD = KB * P
GPAIRS = KB // 2                 # 40 DoubleRow pairs
ASCALE = 64.0                    # fp8 pre-scale on A (undone downstream)
F8 = mybir.dt.float8e4
F16 = mybir.dt.float16
F32 = mybir.dt.float32

# free-axis chunks of the local dst range (PSUM bank = 512 f32);
# chunk i covers dst blocks CH_BLK[i]
N_CHUNKS = [(0, 512), (512, 1024), (1024, NB)]
CH_BLK = [(0, 4), (4, 8), (8, 10)]
# y all-gather split: chunk a = blocks 0-5, chunk b = blocks 6-9
AG_SPLIT = 6 * D_OUT             # 384 cols of y8l
A_PAIRS = [g for g in range(GPAIRS) if g % 5 < 3]   # pairs served by AG a
B_PAIRS = [g for g in range(GPAIRS) if g % 5 >= 3]  # pairs served by AG b

_compiled_nc = None
LAST_EXEC_NS = None


def _build_nc():
    nc = bacc.Bacc("TRN2", target_bir_lowering=False, debug=False,
                   num_devices=N_CORES)

    as_d = nc.dram_tensor("as8", [GPAIRS, P, 2 * NB], F8, kind="ExternalInput")
    hs_d = nc.dram_tensor("hsb8", [P, KB * D_IN], F8, kind="ExternalInput")
    ht_d = nc.dram_tensor("ht", [D_IN, NB], F16, kind="ExternalInput")
    w1_d = nc.dram_tensor("w1", [2 * D_IN, D_HID], F16, kind="ExternalInput")
    w2a_d = nc.dram_tensor("w2a", [D_HID, D_OUT], F16, kind="ExternalInput")
    w2b_d = nc.dram_tensor("w2b", [D_HID, D_OUT], F16, kind="ExternalInput")
    b1_d = nc.dram_tensor("b1c", [P, 2], F32, kind="ExternalInput")
    b2_d = nc.dram_tensor("b2c", [D_OUT, 1], F32, kind="ExternalInput")
    out_d = nc.dram_tensor("outT", [D_OUT, NB], F32, kind="ExternalOutput")

    with tile.TileContext(nc) as tc:
        with (
            tc.tile_pool(name="const", bufs=1) as cpool,
            tc.tile_pool(name="acache", bufs=1) as acpool,
            tc.tile_pool(name="work", bufs=1) as wpool,
            tc.tile_pool(name="dram", bufs=1, space="DRAM") as dpool,
        ):
            # ---- warm-up collectives at t~0 (gpsimd-only chain): the first
            # absorbs the one-time collective-engine init, the second the
            # per-collective rendezvous, so the real all-gathers below start
            # within ~1us of their trigger.
            warm_sb = cpool.tile([1, 32], F8)
            nc.vector.memset(warm_sb[:], 0.0)
            warm_in = dpool.tile([1, 32], F8)
            warm_out = dpool.tile([N_CORES, 32], F8, addr_space="Shared")
            warm_out2 = dpool.tile([N_CORES, 32], F8, addr_space="Shared")
            nc.sync.dma_start(out=warm_in[:], in_=warm_sb[:])
            for wo in (warm_out, warm_out2):
                nc.gpsimd.collective_compute(
                    "AllGather", mybir.AluOpType.bypass,
                    replica_groups=[list(range(N_CORES))],
                    ins=[warm_in[:]], outs=[wo[:]])

            # ---- resident loads ------------------------------------------
            hsb = cpool.tile([P, KB * D_IN], F8)
            HC = KB * D_IN // 4
            for j in range(4):
                nc.gpsimd.dma_start(out=hsb[:, j * HC:(j + 1) * HC],
                                    in_=hs_d[:, j * HC:(j + 1) * HC])
            hts = cpool.tile([P, NB], F16)
            nc.gpsimd.dma_start(out=hts[:], in_=ht_d[:])
            w1s = cpool.tile([P, 2 * D_HID], F16)
            for k in range(2):
                nc.gpsimd.dma_start(out=w1s[:, k * D_HID:(k + 1) * D_HID],
                                    in_=w1_d[k * P:(k + 1) * P, :])
            w2as = cpool.tile([P, 2 * D_OUT], F16)
            w2bs = cpool.tile([P, 2 * D_OUT], F16)
            for k in range(2):
                nc.gpsimd.dma_start(out=w2as[:, k * D_OUT:(k + 1) * D_OUT],
                                    in_=w2a_d[k * P:(k + 1) * P, :])
                nc.gpsimd.dma_start(out=w2bs[:, k * D_OUT:(k + 1) * D_OUT],
                                    in_=w2b_d[k * P:(k + 1) * P, :])
            b1s = cpool.tile([P, 2], F32)
            nc.gpsimd.dma_start(out=b1s[:], in_=b1_d[:])
            b2s = cpool.tile([D_OUT, 1], F32)
            nc.gpsimd.dma_start(out=b2s[:], in_=b2_d[:])

            # A^T stream: 40 pair tiles of [128, 2*1250] fp8, striped over
            # all three DMA-capable queues so delivery tracks the k-sweep.
            acq = [acpool.tile([P, 2 * NB], F8, name=f"acq{q}")
                   for q in range(GPAIRS)]
            for q in range(GPAIRS):
                eng = (nc.sync, nc.scalar, nc.gpsimd)[q % 3]
                eng.dma_start(out=acq[q][:], in_=as_d[q])

            def a_pair(q):
                return acq[q][:].rearrange("p (two d) -> p two d", two=2)

            hNT = wpool.tile([P, NB], F16)
            # xT padded to 1280 cols; the 30-col tail is zeroed once so the
            # ragged last y-block can matmul a full 128-wide stationary.
            xT = [wpool.tile([P, BPC * P], F16, name=f"xT{m}") for m in range(2)]
            for m in range(2):
                nc.vector.memset(xT[m][:, NB:], 0.0)
            y8l = wpool.tile([P, BPC * D_OUT], F8)
            y8g = wpool.tile([P, KB * D_OUT], F8)
            outsb = wpool.tile([D_OUT, NB], F32)

            # ---- layer 1 aggregation: hN^T = sum_k hk^T . As_k (x64) -----
            # fp8 DoubleRow over all 40 src pairs
            with tc.tile_pool(name="ps1", bufs=1, space="PSUM") as ps1:
                hN_ps = ps1.tile([P, NB], F32, space="PSUM")
                for q in range(GPAIRS):
                    lhs_pair = hsb[:, q * 2 * D_IN:(q + 1) * 2 * D_IN] \
                        .rearrange("p (two f) -> p two f", two=2)
                    rhs_pair = a_pair(q)
                    for (n0, n1) in N_CHUNKS:
                        nc.tensor.matmul(
                            out=hN_ps[:, n0:n1],
                            lhsT=lhs_pair,
                            rhs=rhs_pair[:, :, n0:n1],
                            perf_mode=mybir.MatmulPerfMode.DoubleRow,
                            start=(q == 0), stop=(q == GPAIRS - 1))
                # pure copy: the 1/64 is folded into W1's hN-half on host
                for i, (n0, n1) in enumerate(N_CHUNKS):
                    if i == 1:
                        nc.vector.tensor_copy(out=hNT[:, n0:n1],
                                              in_=hN_ps[:, n0:n1])
                    else:
                        nc.scalar.activation(
                            out=hNT[:, n0:n1], in_=hN_ps[:, n0:n1],
                            func=mybir.ActivationFunctionType.Copy)

            # ---- per-chunk pipelined chain: linear -> relu -> y -> AG ----
            # x^T = relu(W1^T . [h; hN]^T + b1); y = x @ W2b row-major;
            # o_ps also picks up its pre-AG W2a part per chunk.
            cat1 = [hts, hNT]
            ag_in_a = dpool.tile([P, AG_SPLIT], F8)
            ag_in_b = dpool.tile([P, BPC * D_OUT - AG_SPLIT], F8)
            ag_out_a = dpool.tile([N_CORES * P, AG_SPLIT], F8,
                                  addr_space="Shared")
            ag_out_b = dpool.tile([N_CORES * P, BPC * D_OUT - AG_SPLIT], F8,
                                  addr_space="Shared")

            ps2_ctx = tc.tile_pool(name="ps2", bufs=2, space="PSUM")
            ps2 = ps2_ctx.__enter__()
            ps3_ctx = tc.tile_pool(name="ps3", bufs=1, space="PSUM")
            ps3 = ps3_ctx.__enter__()
            ps4_ctx = tc.tile_pool(name="ps4", bufs=1, space="PSUM")
            ps4 = ps4_ctx.__enter__()
            o_ps = ps4.tile([D_OUT, NB], F32, space="PSUM")

            for ci, (n0, n1) in enumerate(N_CHUNKS):
                cw = n1 - n0
                # layer-1 linear for this chunk (m halves side by side in
                # one 2-bank ring tile)
                y_ps = ps2.tile([P, 1024], F32, space="PSUM", name="y_ps")
                for m in range(2):
                    for k in range(2):
                        nc.tensor.matmul(
                            out=y_ps[:, m * 512:m * 512 + cw],
                            lhsT=w1s[:, k * D_HID + m * P:
                                     k * D_HID + (m + 1) * P],
                            rhs=cat1[k][:, n0:n1],
                            start=(k == 0), stop=(k == 1))
                for m in range(2):
                    nc.scalar.activation(
                        out=xT[m][:, n0:n1],
                        in_=y_ps[:, m * 512:m * 512 + cw],
                        func=mybir.ActivationFunctionType.Relu,
                        bias=b1s[:, m:m + 1])
                # y blocks of this chunk (row-major, x^T-stationary)
                b0, b1_ = CH_BLK[ci]
                y_ps2 = ps3.tile([P, 4 * D_OUT], F32, space="PSUM",
                                 name="y_ps2")
                for bi, b in enumerate(range(b0, b1_)):
                    for m in range(2):
                        nc.tensor.matmul(
                            out=y_ps2[:, bi * D_OUT:(bi + 1) * D_OUT],
                            lhsT=xT[m][:, b * P:(b + 1) * P],
                            rhs=w2bs[:, m * D_OUT:(m + 1) * D_OUT],
                            start=(m == 0), stop=(m == 1))
                nc.vector.tensor_copy(
                    out=y8l[:, b0 * D_OUT:b1_ * D_OUT],
                    in_=y_ps2[:, :(b1_ - b0) * D_OUT])
                # pre-AG W2a part of the output (W2a pre-scaled x64)
                for m in range(2):
                    nc.tensor.matmul(
                        out=o_ps[:, n0:n1],
                        lhsT=w2as[:, m * D_OUT:(m + 1) * D_OUT],
                        rhs=xT[m][:, n0:n1],
                        start=(m == 0), stop=False)
                # kick off each all-gather as soon as its y columns exist
                if ci == 1:
                    nc.gpsimd.dma_start(out=ag_in_a[:],
                                        in_=y8l[:, :AG_SPLIT])
                    nc.gpsimd.collective_compute(
                        "AllGather", mybir.AluOpType.bypass,
                        replica_groups=[list(range(N_CORES))],
                        ins=[ag_in_a[:]], outs=[ag_out_a[:]])
                elif ci == 2:
                    nc.gpsimd.dma_start(out=ag_in_b[:],
                                        in_=y8l[:, AG_SPLIT:])
                    nc.gpsimd.collective_compute(
                        "AllGather", mybir.AluOpType.bypass,
                        replica_groups=[list(range(N_CORES))],
                        ins=[ag_in_b[:]], outs=[ag_out_b[:]])

            # ---- layer 2 sweep: out^T += sum_k yk^T . As_k (x64) ---------
            # a-pairs (y blocks 0-3 of every region) run while AG b is
            # still in flight.
            for c in range(N_CORES):
                nc.sync.dma_start(
                    out=y8g[:, c * BPC * D_OUT:c * BPC * D_OUT + AG_SPLIT],
                    in_=ag_out_a[c * P:(c + 1) * P, :])
            for c in range(N_CORES):
                nc.scalar.dma_start(
                    out=y8g[:, c * BPC * D_OUT + AG_SPLIT:
                            (c + 1) * BPC * D_OUT],
                    in_=ag_out_b[c * P:(c + 1) * P, :])
            # a-pairs for all chunks run while AG b is in flight; then the
            # b-pairs go chunk-major so each chunk's output tail (ACT + DMA)
            # pipelines under the next chunk's matmuls.
            for g in A_PAIRS:
                lhs_pair = y8g[:, g * 2 * D_OUT:(g + 1) * 2 * D_OUT] \
                    .rearrange("p (two f) -> p two f", two=2)
                rhs_pair = a_pair(g)
                for (n0, n1) in N_CHUNKS:
                    nc.tensor.matmul(
                        out=o_ps[:, n0:n1],
                        lhsT=lhs_pair,
                        rhs=rhs_pair[:, :, n0:n1],
                        perf_mode=mybir.MatmulPerfMode.DoubleRow,
                        start=False, stop=False)
            for (n0, n1) in N_CHUNKS:
                for g in B_PAIRS:
                    lhs_pair = y8g[:, g * 2 * D_OUT:(g + 1) * 2 * D_OUT] \
                        .rearrange("p (two f) -> p two f", two=2)
                    rhs_pair = a_pair(g)
                    nc.tensor.matmul(
                        out=o_ps[:, n0:n1],
                        lhsT=lhs_pair,
                        rhs=rhs_pair[:, :, n0:n1],
                        perf_mode=mybir.MatmulPerfMode.DoubleRow,
                        start=False, stop=(g == B_PAIRS[-1]))
                nc.scalar.activation(out=outsb[:, n0:n1],
                                     in_=o_ps[:, n0:n1],
                                     func=mybir.ActivationFunctionType.Identity,
                                     bias=b2s[:, 0:1],
                                     scale=1.0 / ASCALE)
                nc.sync.dma_start(out=out_d[:, n0:n1],
                                  in_=outsb[:, n0:n1])

            ps4_ctx.__exit__(None, None, None)
            ps3_ctx.__exit__(None, None, None)
            ps2_ctx.__exit__(None, None, None)

    nc.compile()
    return nc


def _get_nc():
    global _compiled_nc
    if _compiled_nc is None:
        _compiled_nc = _build_nc()
    return _compiled_nc


def _enable_profile_hook():
    """Register the NTFF profiling hook that trn_boot skips when the image's
    antenv lacks axon_hooks (profiling only; used when GNN_PROFILE=1)."""
    try:
        import antenv
        if "antenv.axon_hooks" not in sys.modules:
            mod = types.ModuleType("antenv.axon_hooks")
            _h = [None]
            mod.set_axon_ntff_profile_hook = lambda hook: _h.__setitem__(0, hook)
            mod.get_axon_ntff_profile_hook = lambda: _h[0]
            sys.modules["antenv.axon_hooks"] = mod
            antenv.axon_hooks = mod
        from trn_agent_boot.trn_boot import _ntff_profile_via_ctypes
        hook = _ntff_profile_via_ctypes("/opt/axon/libaxon_pjrt.so")
        if hook is not None:
            sys.modules["antenv.axon_hooks"].set_axon_ntff_profile_hook(hook)
            return True
    except Exception:
        pass
    return False


def _host_prep(h, w, src, dst, W1, b1, W2, b2):
    import ml_dtypes
    import scipy.sparse as sp
    deg = np.bincount(dst, minlength=N_NODES).astype(np.float32)
    w_norm = (w[:, 0] * (ASCALE / np.maximum(deg, 1.0)[dst])).astype(np.float32)
    # AT[s, d] = sum of scaled w_norm over edges (s -> d): 64*A^T
    AT = sp.coo_matrix((w_norm, (src, dst)), shape=(N_NODES, N_NODES)).toarray()

    # per-core-padded src layout: row c*1280 + j <-> node c*1250 + j (j<1250)
    ATp = np.zeros((NPAD, N_NODES), dtype=np.float32)
    hp = np.zeros((NPAD, D_IN), dtype=np.float32)
    for c in range(N_CORES):
        ATp[c * BPC * P: c * BPC * P + NB] = AT[c * NB:(c + 1) * NB]
        hp[c * BPC * P: c * BPC * P + NB] = h[c * NB:(c + 1) * NB]
    AT8 = ATp.astype(ml_dtypes.float8_e4m3)
    hp8 = hp.astype(ml_dtypes.float8_e4m3)

    # hsb8[p, k*128+f] = hp8[k*128+p, f] (global padded block order)
    hsb8 = np.ascontiguousarray(
        hp8.reshape(KB, P, D_IN).transpose(1, 0, 2).reshape(P, KB * D_IN))

    w1c = W1.astype(np.float32).copy()
    w1c[D_IN:] *= 1.0 / ASCALE        # hN carries x64 out of its PSUM copy
    w1c = w1c.astype(np.float16)
    w2a = (W2[:D_HID] * ASCALE).astype(np.float16)
    w2b = W2[D_HID:].astype(np.float16)
    b1c = np.ascontiguousarray(b1.reshape(2, P).T.astype(np.float32))
    b2c = b2.reshape(D_OUT, 1).astype(np.float32)

    in_maps = []
    for c in range(N_CORES):
        ATc = AT8[:, c * NB:(c + 1) * NB]
        # as8[q, p, j*NB+d] = ATc[(2q+j)*128+p, d] (pair-interleaved)
        as8 = np.ascontiguousarray(
            ATc.reshape(GPAIRS, 2, P, NB).transpose(0, 2, 1, 3)
            .reshape(GPAIRS, P, 2 * NB))
        in_maps.append({
            "as8": as8,
            "hsb8": hsb8,
            "ht": np.ascontiguousarray(
                h[c * NB:(c + 1) * NB].T.astype(np.float16)),
            "w1": w1c,
            "w2a": w2a,
            "w2b": w2b,
            "b1c": b1c,
            "b2c": b2c,
        })
    return in_maps


def kernel(h, w, src, dst, W1, b1, W2, b2):
    global LAST_EXEC_NS
    h = np.asarray(h, dtype=np.float32)
    w = np.asarray(w, dtype=np.float32)
    src = np.asarray(src)
    dst = np.asarray(dst)
    W1 = np.asarray(W1, dtype=np.float32)
    b1 = np.asarray(b1, dtype=np.float32)
    W2 = np.asarray(W2, dtype=np.float32)
    b2 = np.asarray(b2, dtype=np.float32)

    in_maps = _host_prep(h, w, src, dst, W1, b1, W2, b2)
    nc = _get_nc()
    trace = os.environ.get("GNN_PROFILE") == "1" and _enable_profile_hook()
    res = bass_utils.run_bass_kernel_spmd(
        nc, in_maps, core_ids=list(range(N_CORES)), trace=trace)
    LAST_EXEC_NS = res.exec_time_ns

    out = np.concatenate(
        [res.results[c]["outT"].T for c in range(N_CORES)], axis=0)
    return out.astype(np.float32)
